# revision 1
# baseline (speedup 1.0000x reference)
"""Trainium2 Bass kernel for ContrastiveAffinityLossWithMemoryV2.

Math: with MARGIN=4 and d = ||a-b|| <= 2 for unit vectors, relu(M-d) = M-d,
so each pairwise loss term simplifies:
    t*d^2 + (1-t)*(M-d)^2 = d^2 + (1-t)*(16 - 8*d)
Sum(d^2) and Sum(1-t) are *linear* and evaluated exactly on host from vector
sums; the only part needing the full B x B pair plane / B x C memory plane is
    P3 = Sum 8*d * (1-t)
which the device computes, sharded over 8 NeuronCores:
  - PE: psum = -2*S (bf16 operands pre-scaled by -2, fp32 accumulate)
  - ScalarE: d8 = sqrt(64*psum + 128 + delta) = 8*d   (the "+2" constant is
    supplied via the activation bias; delta keeps the arg positive, which the
    host guarantees by truncating embeddings to bf16 *toward zero* so every
    row norm stays <= 1)
  - VectorE: scalar_tensor_tensor fused multiply+reduce against host-shipped
    fp8 masks (stochastically rounded so quantization is unbiased), giving
    per-partition partial sums.
The pair plane is computed only for j > i: row-blocks are dealt to cores so
every core owns exactly 18 of the 144 upper-triangle (row-block x 512-chunk)
units; per-unit operands are duplicated into flat arrays so all cores run the
same program on different data.  Host combines partials with the closed-form
terms.
"""

import numpy as np
import ml_dtypes

N_CLASSES = 8192
B = 4096
D = 192  # 256 * 0.75
NCORES = 8
ROWS = B // NCORES          # 512 rows per core
NRB = B // 128              # 32 global row-blocks
MARGIN = 4.0
MEMORY_WEIGHT = 0.5
WARMUP_STEPS = 1000
MOM_WARMUP = 5000
BASE_MOM = 0.9
BG_SIM = 0.2
BG_OTHER_SIM = 0.01
EPS = 1e-12
DELTA2 = 0.01
NGU = 18                    # G-plane units per core (144 / 8)

bf16 = ml_dtypes.bfloat16
f8 = ml_dtypes.float8_e4m3

# row-block deal: cores 0-3 get chunk-counts {8,7,2,1}, cores 4-7 {6,5,4,3}
CORE_RBS = [[k, 4 + k, 24 + k, 28 + k] for k in range(4)] + \
           [[8 + k, 12 + k, 16 + k, 20 + k] for k in range(4)]


def _g_chunks(rb):
    """512-col chunks containing any j > i for row-block rb."""
    return [cc for cc in range(8) if 512 * cc + 511 >= 128 * rb + 1]


_CACHE = {}


def cap_bf16(v):
    """fp32 -> bf16 truncated toward zero: row L2 norms can only shrink."""
    x = np.ascontiguousarray(v, dtype=np.float32)
    return (x.view(np.uint32) >> 16).astype(np.uint16).view(bf16)


def stoch_fp8(v, seed):
    """Stochastic rounding to float8_e4m3 (values >= 0)."""
    x = np.ascontiguousarray(v, dtype=np.float32)
    y = x.astype(f8)
    yb = y.view(np.uint8).copy()
    over = np.abs(y.astype(np.float32)) > x
    yb[over & ((yb & 0x7F) > 0)] -= 1
    fl = yb.view(f8)
    ce = (yb + (fl.astype(np.float32) < x).astype(np.uint8)).view(f8)
    flf = fl.astype(np.float32)
    gap = ce.astype(np.float32) - flf
    p = np.where(gap > 0, (x - flf) / np.where(gap > 0, gap, 1.0), 0.0)
    rng = np.random.default_rng(seed)
    up = rng.random(x.shape, dtype=np.float32) < p
    return np.where(up, ce, fl).astype(f8)


def _bank_chains(zn, y_true, momentum):
    """Replicate the reference's sequential per-sample EMA scatter (fp32)."""
    valid = (y_true >= 0) & (y_true < N_CLASSES)
    lc = np.clip(y_true, 0, N_CLASSES - 1)
    m = np.float32(momentum)
    one_m = np.float32(1.0 - momentum)
    bank = {}
    for i in np.nonzero(valid)[0]:
        c = int(lc[i])
        if c not in bank:
            bank[c] = zn[i].copy()
        else:
            ema = m * bank[c] + one_m * zn[i]
            n = np.float32(np.sqrt(np.float32((ema ** 2).sum())))
            bank[c] = ema / max(n, np.float32(EPS))
    return bank


def _build_nc(CS):
    """CS = number of 512-wide S-plane chunks (CP = 512*CS classes)."""
    from concourse import bacc, tile, mybir

    dt = mybir.dt
    CP = 512 * CS
    nc = bacc.Bacc("TRN2", target_bir_lowering=False, debug=False)

    lhsA_d = nc.dram_tensor("lhsA", (128, ROWS), dt.bfloat16, kind="ExternalInput")
    lhsB_d = nc.dram_tensor("lhsB", (64, ROWS), dt.bfloat16, kind="ExternalInput")
    rsA_d = nc.dram_tensor("rsA", (128, CP), dt.bfloat16, kind="ExternalInput")
    rsB_d = nc.dram_tensor("rsB", (64, CP), dt.bfloat16, kind="ExternalInput")
    lgA_d = nc.dram_tensor("lgA", (128, NGU * 128), dt.bfloat16, kind="ExternalInput")
    lgB_d = nc.dram_tensor("lgB", (64, NGU * 128), dt.bfloat16, kind="ExternalInput")
    rgA_d = nc.dram_tensor("rgA", (128, NGU * 512), dt.bfloat16, kind="ExternalInput")
    rgB_d = nc.dram_tensor("rgB", (64, NGU * 512), dt.bfloat16, kind="ExternalInput")
    r1_d = nc.dram_tensor("r1", (128, 4 * CP), dt.float8e4, kind="ExternalInput")
    t2_d = nc.dram_tensor("t2", (128, NGU * 512), dt.float8e4, kind="ExternalInput")
    out_d = nc.dram_tensor("acc_out", (128, 16), dt.float32, kind="ExternalOutput")

    # unit list: (lhs tensor key, lhs col, rhs key, rhs col, mask key, mask col)
    units = []
    for ib in range(4):
        for cc in range(CS):
            units.append(("s", ib * 128, cc * 512, (ib * CS + cc) * 512))
    for u in range(NGU):
        units.append(("g", u * 128, u * 512, u * 512))
    n_units = len(units)
    n_groups = (n_units + 3) // 4
    assert n_groups <= 16

    DMA_SPLIT = 4  # split big resident tensors into this many DMAs

    with tile.TileContext(nc) as tc:
        with (
            tc.tile_pool(name="const", bufs=1) as constp,
            tc.tile_pool(name="d8p", bufs=3) as d8p,
            tc.tile_pool(name="ep", bufs=2) as ep,
            tc.tile_pool(name="accp", bufs=1) as accp,
            tc.tile_pool(name="psp", bufs=2, space="PSUM") as psp,
        ):
            def load(dram, shape, dtype, name, split=DMA_SPLIT):
                t = constp.tile(list(shape), dtype, tag=name)
                w = shape[1] // split
                for s in range(split):
                    nc.sync.dma_start(
                        t[:, s * w:(s + 1) * w], dram[:, s * w:(s + 1) * w]
                    )
                return t

            lhsA = load(lhsA_d, (128, ROWS), dt.bfloat16, "lhsA", 1)
            lhsB = load(lhsB_d, (64, ROWS), dt.bfloat16, "lhsB", 1)
            rsA = load(rsA_d, (128, CP), dt.bfloat16, "rsA", CS)
            rsB = load(rsB_d, (64, CP), dt.bfloat16, "rsB", CS)
            lgA = load(lgA_d, (128, NGU * 128), dt.bfloat16, "lgA", 6)
            lgB = load(lgB_d, (64, NGU * 128), dt.bfloat16, "lgB", 6)
            rgA = load(rgA_d, (128, NGU * 512), dt.bfloat16, "rgA", 6)
            rgB = load(rgB_d, (64, NGU * 512), dt.bfloat16, "rgB", 6)
            r1 = load(r1_d, (128, 4 * CP), dt.float8e4, "r1", 4)
            t2 = load(t2_d, (128, NGU * 512), dt.float8e4, "t2", 6)

            bias_t = constp.tile([128, 1], dt.float32)
            nc.gpsimd.memset(bias_t[:], 128.0 + float(DELTA2))

            acc_all = accp.tile([128, 16], dt.float32)
            nc.gpsimd.memset(acc_all[:], 0.0)

            ops = {"s": (lhsA, lhsB, rsA, rsB, r1), "g": (lgA, lgB, rgA, rgB, t2)}
            for gi in range(n_groups):
                gunits = units[gi * 4:(gi + 1) * 4]
                gw = 512 * len(gunits)
                ps = psp.tile([128, 2048], dt.float32, tag="ps")
                for q, (key, lc0, rc0, mc0) in enumerate(gunits):
                    lA, lB, rA, rB, _ = ops[key]
                    o = ps[:, q * 512:(q + 1) * 512]
                    nc.tensor.matmul(
                        o, lA[:, lc0:lc0 + 128], rA[:, rc0:rc0 + 512],
                        start=True, stop=False,
                    )
                    nc.tensor.matmul(
                        o, lB[:, lc0:lc0 + 128], rB[:, rc0:rc0 + 512],
                        start=False, stop=True,
                    )
                d8 = d8p.tile([128, 2048], dt.bfloat16, tag="d8")
                nc.scalar.activation(
                    d8[:, 0:gw], ps[:, 0:gw],
                    mybir.ActivationFunctionType.Sqrt,
                    bias=bias_t[:], scale=64.0,
                )
                et = ep.tile([128, 2048], dt.bfloat16, tag="et")
                # all units in a group share one mask tensor and their mask
                # columns are consecutive by construction
                mkey, mc0 = gunits[0][0], gunits[0][3]
                mask = ops[mkey][4]
                nc.vector.scalar_tensor_tensor(
                    out=et[:, 0:gw],
                    in0=d8[:, 0:gw],
                    scalar=1.0,
                    in1=mask[:, mc0:mc0 + gw],
                    op0=mybir.AluOpType.mult,
                    op1=mybir.AluOpType.mult,
                    accum_out=acc_all[:, gi:gi + 1],
                )

            nc.sync.dma_start(out_d[:], acc_all[:])

    nc.compile()
    n_groups_s = (4 * CS + 3) // 4
    return nc, n_groups, n_groups_s


def _get_nc(CS):
    key = ("nc", CS)
    if key not in _CACHE:
        _CACHE[key] = _build_nc(CS)
    return _CACHE[key]


def kernel(y_true, y_pred, lookup, global_step, current_epoch, _want_trace=False):
    from concourse.bass_utils import run_bass_kernel_spmd

    y_true = np.asarray(y_true).astype(np.int64)
    y_pred = np.asarray(y_pred, dtype=np.float32)
    lookup = np.asarray(lookup, dtype=np.float32)
    gs = int(np.asarray(global_step))

    if gs < MOM_WARMUP:
        momentum = 0.5 + (BASE_MOM - 0.5) * (gs / MOM_WARMUP)
    else:
        momentum = BASE_MOM
    progress = min(1.0, (gs - WARMUP_STEPS) / 5000.0)
    aw = MEMORY_WEIGHT * progress

    # ---- host: normalize, bank scatter-EMA, compaction ----
    z = y_pred[:, :D]
    nrm = np.sqrt((z.astype(np.float64) ** 2).sum(axis=1))
    zn = (z / np.maximum(nrm, EPS)[:, None]).astype(np.float32)

    valid = (y_true >= 0) & (y_true < N_CLASSES)
    bg = ~valid
    nv = int(valid.sum())
    lc = np.clip(y_true, 0, N_CLASSES - 1)

    bank = _bank_chains(zn, y_true, momentum)
    init_list = np.array(sorted(bank.keys()), dtype=np.int64)
    C = len(init_list)
    CS = max(1, (C + 511) // 512)
    CP = 512 * CS

    zn_bf = cap_bf16(zn)
    bank_rows = (
        np.stack([bank[c] for c in init_list])
        if C else np.zeros((0, D), np.float32)
    )
    bank_bf = cap_bf16(bank_rows)

    znd = zn_bf.astype(np.float64)
    bankd = bank_bf.astype(np.float64)

    # ---- host: exact linear terms (fp64) ----
    R = lookup[lc]                    # (B, 8192)
    R_init = R[:, init_list]          # (B, C)
    A_S = 2.0 * nv * C - 2.0 * float(znd[valid].sum(0) @ bankd.sum(0))
    B_S = nv * C - float(R_init[valid].sum(dtype=np.float64))

    T_up = R[:, lc]                   # (B, B): lookup[lc_i, lc_j]
    both_bg = bg[:, None] & bg[None, :]
    one_bg = bg[:, None] ^ bg[None, :]
    T_up = np.where(both_bg, np.float32(BG_SIM),
                    np.where(one_bg, np.float32(BG_OTHER_SIM), T_up))
    # upper-triangle (i<j) oriented pair targets; zero elsewhere
    T_up = np.triu(T_up, 1)

    Np = B * (B - 1) // 2
    szn = znd.sum(0)
    sumG_offdiag = float(szn @ szn) - float((znd ** 2).sum())
    A_G = 2.0 * Np - sumG_offdiag
    B_G = Np - float(T_up.sum(dtype=np.float64))

    # ---- device operand construction ----
    znT = np.ascontiguousarray(zn_bf.T)                     # (192, B)
    znTm2 = np.ascontiguousarray(
        (zn_bf.astype(np.float32).T * np.float32(-2.0)).astype(bf16)
    )
    bankTm2 = np.zeros((D, CP), dtype=bf16)
    if C:
        bankTm2[:, 0:C] = (
            bank_bf.astype(np.float32).T * np.float32(-2.0)
        ).astype(bf16)

    # triangle mask base: (1 - t_up) with 0 at/below diagonal, bg handled,
    # valid rows only for the S plane
    in_maps = []
    for core in range(NCORES):
        rbs = CORE_RBS[core]
        rows = np.concatenate([np.arange(rb * 128, rb * 128 + 128) for rb in rbs])

        lhs = znT[:, rows]                                  # (192, 512)
        r1 = np.zeros((128, 4 * CP), dtype=f8)
        for ib, rb in enumerate(rbs):
            rr = slice(rb * 128, rb * 128 + 128)
            m = (1.0 - R_init[rr]) * valid[rr, None]        # (128, C)
            r1[:, ib * CP:ib * CP + C] = stoch_fp8(m, seed=1000 + rb)

        gunits = [(ib, rb, cc) for ib, rb in enumerate(rbs)
                  for cc in _g_chunks(rb)]
        assert len(gunits) == NGU, (core, len(gunits))

        lg = np.empty((D, NGU * 128), dtype=bf16)
        rg = np.empty((D, NGU * 512), dtype=bf16)
        t2 = np.zeros((128, NGU * 512), dtype=f8)
        for u, (ib, rb, cc) in enumerate(gunits):
            lg[:, u * 128:(u + 1) * 128] = znT[:, rb * 128:rb * 128 + 128]
            rg[:, u * 512:(u + 1) * 512] = znTm2[:, cc * 512:(cc + 1) * 512]
            blk = 1.0 - T_up[rb * 128:rb * 128 + 128, cc * 512:(cc + 1) * 512]
            jj = np.arange(cc * 512, cc * 512 + 512)[None, :]
            ii = np.arange(rb * 128, rb * 128 + 128)[:, None]
            blk = np.where(jj > ii, blk, 0.0)
            t2[:, u * 512:(u + 1) * 512] = stoch_fp8(blk, seed=2000 + rb * 8 + cc)

        in_maps.append({
            "lhsA": np.ascontiguousarray(lhs[0:128]),
            "lhsB": np.ascontiguousarray(lhs[128:192]),
            "rsA": np.ascontiguousarray(bankTm2[0:128]),
            "rsB": np.ascontiguousarray(bankTm2[128:192]),
            "lgA": np.ascontiguousarray(lg[0:128]),
            "lgB": np.ascontiguousarray(lg[128:192]),
            "rgA": np.ascontiguousarray(rg[0:128]),
            "rgB": np.ascontiguousarray(rg[128:192]),
            "r1": r1,
            "t2": t2,
        })

    nc, n_groups, n_groups_s = _get_nc(CS)
    if _want_trace:
        import tempfile
        try:
            from trn_agent_boot.trn_boot import _ntff_profile_via_ctypes
            hook = _ntff_profile_via_ctypes("/opt/axon/libaxon_pjrt.so")
            outdir = tempfile.mkdtemp(prefix="ntff_")
            with hook(outdir, [0]):
                res = run_bass_kernel_spmd(nc, in_maps, list(range(NCORES)))
            _CACHE["last_profile_dir"] = outdir
        except Exception as e:
            _CACHE["trace_error"] = repr(e)
            res = run_bass_kernel_spmd(nc, in_maps, list(range(NCORES)))
        _CACHE["last_results"] = res
    else:
        res = run_bass_kernel_spmd(nc, in_maps, list(range(NCORES)))

    P3S = 0.0
    P3G = 0.0
    for r in res.results:
        acc = np.asarray(r["acc_out"], dtype=np.float64)
        P3S += float(acc[:, 0:n_groups_s].sum())
        P3G += float(acc[:, n_groups_s:n_groups].sum())

    mem_sum = A_S + 16.0 * B_S - P3S
    denom = max(nv * C, 1)
    mem_loss = mem_sum / denom

    batch_sum = A_G + 16.0 * B_G - P3G
    batch_loss = batch_sum / Np

    loss = (1.0 - aw) * batch_loss + aw * mem_loss
    return np.float32(loss)



# revision 3
# speedup vs baseline: 1.6390x; 1.6390x over previous
"""Trainium2 Bass kernel for ContrastiveAffinityLossWithMemoryV2.

Math: with MARGIN=4 and d = ||a-b|| <= 2 for unit vectors, relu(M-d) = M-d,
so each pairwise loss term simplifies:
    t*d^2 + (1-t)*(M-d)^2 = d^2 + (1-t)*(16 - 8*d)
Sum(d^2) and Sum(1-t) are *linear* and evaluated exactly on host from vector
sums; the only part needing the full B x B pair plane / B x C memory plane is
    P3 = Sum 8*d * (1-t)
which the device computes, sharded over 8 NeuronCores:
  - PE: psum = S via fp8 e4m3 DoubleRow matmuls (K=192 packed as 2 k-tiles of
    96 partitions; one matmul per 128x512 unit at 0.5 cycles/row)
  - ScalarE: d8 = sqrt(-128*psum + 128 + delta) = 8*d (negative scale folds
    the "2-2S" form into the activation's free affine; embeddings/bank rows
    are truncated toward zero in fp8 so every norm stays <= 1 and the sqrt
    argument stays positive)
  - VectorE: scalar_tensor_tensor fused multiply+reduce against host-shipped
    fp8 masks (stochastically rounded so quantization is unbiased), giving
    per-partition partial sums.

The pair plane is computed only for j > i. All cores run ONE program with a
fixed slot pattern; per-core differences live entirely in the data: each
core's znp tensor is [8 rhs windows x 512 cols | 6 lhs blocks x 128 cols]
where the host fills window w with a 512-col chunk and lhs slot p with a
128-row block of its choice. Cores 0-3 have chunk-count profile {8,7,2,1},
cores 4-7 {6,5,4,3}; both embed exactly into the fixed pattern whose
window slot counts are {4,3,2,2,2,2,2,1}.  Host combines device partials
with the closed-form terms.
"""

import numpy as np
import ml_dtypes

N_CLASSES = 8192
B = 4096
D = 192  # 256 * 0.75
KP = 96  # partitions per k-tile (2 tiles of 96 = 192)
NCORES = 8
MARGIN = 4.0
MEMORY_WEIGHT = 0.5
WARMUP_STEPS = 1000
MOM_WARMUP = 5000
BASE_MOM = 0.9
BG_SIM = 0.2
BG_OTHER_SIM = 0.01
EPS = 1e-12
DELTA2 = 0.01
NGU = 18                    # G-plane units per core (144 / 8)
GW = 8 * 512                # G-rhs window region width in znp
ZNP_COLS = GW + 6 * 128     # + 6 lhs block slots

bf16 = ml_dtypes.bfloat16
f8 = ml_dtypes.float8_e4m3

# row-block deal: cores 0-3 get chunk-counts {8,7,2,1}, cores 4-7 {6,5,4,3}
CORE_RBS = [[k, 4 + k, 24 + k, 28 + k] for k in range(4)] + \
           [[8 + k, 12 + k, 16 + k, 20 + k] for k in range(4)]

# fixed G-plane slot pattern: (window, lhs slot), ordered so consecutive
# slots share the same lhs block (stationary reuse on the PE)
P_SLOTS = [
    (0, 0), (1, 0), (2, 0), (4, 0), (6, 0), (7, 0),
    (0, 1), (1, 1), (2, 1), (4, 1), (6, 1),
    (0, 2), (1, 2),
    (0, 3),
    (3, 4), (5, 4),
    (3, 5), (5, 5),
]
# per-core window contents (chunk id per window) and lhs slot contents
# (index into CORE_RBS[core] per lhs slot):
W_A = [7, 6, 1, 2, 3, 4, 5, 0]       # cores 0-3
W_B = [5, 4, 6, 6, 7, 7, 3, 2]       # cores 4-7
LHS_A = [0, 1, 2, 3, 0, 1]
LHS_B = [0, 1, 2, 3, 2, 3]


def _g_chunks(rb):
    """512-col chunks containing any j > i for row-block rb."""
    return [cc for cc in range(8) if 512 * cc + 511 >= 128 * rb + 1]


def _core_gunits(core):
    """(rb, cc) per G slot for this core; asserts global exact cover."""
    rbs = CORE_RBS[core]
    W = W_A if core < 4 else W_B
    LHS = LHS_A if core < 4 else LHS_B
    return [(rbs[LHS[lp]], W[w]) for (w, lp) in P_SLOTS]


def _check_cover():
    seen = []
    for core in range(NCORES):
        seen += _core_gunits(core)
    need = [(rb, cc) for rb in range(32) for cc in _g_chunks(rb)]
    assert sorted(seen) == sorted(need), "G-plane cover mismatch"


_check_cover()

_CACHE = {}


def cap_fp8(v):
    """fp32 -> fp8 e4m3 truncated toward zero: row L2 norms can only shrink."""
    x = np.ascontiguousarray(v, dtype=np.float32)
    y = x.astype(f8)
    yb = y.view(np.uint8).copy()
    over = np.abs(y.astype(np.float32)) > np.abs(x)
    yb[over & ((yb & 0x7F) > 0)] -= 1
    return yb.view(f8)


def stoch_fp8(v, seed):
    """Stochastic rounding to float8_e4m3 (values >= 0)."""
    x = np.ascontiguousarray(v, dtype=np.float32)
    y = x.astype(f8)
    yb = y.view(np.uint8).copy()
    over = np.abs(y.astype(np.float32)) > x
    yb[over & ((yb & 0x7F) > 0)] -= 1
    fl = yb.view(f8)
    ce = (yb + (fl.astype(np.float32) < x).astype(np.uint8)).view(f8)
    flf = fl.astype(np.float32)
    gap = ce.astype(np.float32) - flf
    p = np.where(gap > 0, (x - flf) / np.where(gap > 0, gap, 1.0), 0.0)
    rng = np.random.default_rng(seed)
    up = rng.random(x.shape, dtype=np.float32) < p
    return np.where(up, ce, fl).astype(f8)


def _bank_chains(zn, y_true, momentum):
    """Replicate the reference's sequential per-sample EMA scatter (fp32)."""
    valid = (y_true >= 0) & (y_true < N_CLASSES)
    lc = np.clip(y_true, 0, N_CLASSES - 1)
    m = np.float32(momentum)
    one_m = np.float32(1.0 - momentum)
    bank = {}
    for i in np.nonzero(valid)[0]:
        c = int(lc[i])
        if c not in bank:
            bank[c] = zn[i].copy()
        else:
            ema = m * bank[c] + one_m * zn[i]
            n = np.float32(np.sqrt(np.float32((ema ** 2).sum())))
            bank[c] = ema / max(n, np.float32(EPS))
    return bank


def _build_nc(CS):
    """CS = number of 512-wide S-plane chunks (CP = 512*CS classes)."""
    from concourse import bacc, tile, mybir

    dt = mybir.dt
    CP = 512 * CS
    nc = bacc.Bacc("TRN2", target_bir_lowering=False, debug=False)

    znp_d = nc.dram_tensor("znp", (KP, 2 * ZNP_COLS), dt.float8e4, kind="ExternalInput")
    bkp_d = nc.dram_tensor("bkp", (KP, 2 * CP), dt.float8e4, kind="ExternalInput")
    r1_d = nc.dram_tensor("r1", (128, 4 * CP), dt.float8e4, kind="ExternalInput")
    t2_d = nc.dram_tensor("t2", (128, NGU * 512), dt.float8e4, kind="ExternalInput")
    out_d = nc.dram_tensor("acc_out", (128, 16), dt.float32, kind="ExternalOutput")

    # unit list: ("s", lhs slot, bank chunk) | ("g", lhs slot, window)
    units = []
    for ib in range(4):
        for cc in range(CS):
            units.append(("s", ib, cc))
    for (w, lp) in P_SLOTS:
        units.append(("g", lp, w))
    n_units = len(units)
    n_groups = (n_units + 3) // 4
    n_groups_s = (4 * CS + 3) // 4
    assert n_groups <= 16

    with tile.TileContext(nc) as tc:
        with (
            tc.tile_pool(name="const", bufs=1) as constp,
            tc.tile_pool(name="d8p", bufs=3) as d8p,
            tc.tile_pool(name="ep", bufs=2) as ep,
            tc.tile_pool(name="psp", bufs=2, space="PSUM") as psp,
        ):
            # resident operands; [KP, 2, N] with k-tile index in free dim 0
            znp = constp.tile([KP, 2, ZNP_COLS], dt.float8e4, tag="znp")
            bkp = constp.tile([KP, 2, CP], dt.float8e4, tag="bkp")
            r1 = constp.tile([128, 4 * CP], dt.float8e4, tag="r1")
            t2 = constp.tile([128, NGU * 512], dt.float8e4, tag="t2")

            # DMA order: lhs blocks + bank + first mask group first so the
            # pipeline can start; bulk window data + remaining masks after.
            for s in range(2):
                nc.sync.dma_start(znp[:, s, GW:ZNP_COLS],
                                  znp_d[:, s * ZNP_COLS + GW:(s + 1) * ZNP_COLS])
            for s in range(2):
                nc.sync.dma_start(bkp[:, s, :], bkp_d[:, s * CP:(s + 1) * CP])
            for g in range(n_groups_s):
                c0 = g * 2048
                w = min(2048, 4 * CP - c0)
                nc.sync.dma_start(r1[:, c0:c0 + w], r1_d[:, c0:c0 + w])
            for s in range(2):
                nc.sync.dma_start(znp[:, s, 0:GW],
                                  znp_d[:, s * ZNP_COLS:s * ZNP_COLS + GW])
            for g in range(n_groups - n_groups_s):
                c0 = g * 2048
                w = min(2048, NGU * 512 - c0)
                nc.sync.dma_start(t2[:, c0:c0 + w], t2_d[:, c0:c0 + w])

            bias_t = constp.tile([128, 1], dt.float32)
            nc.gpsimd.memset(bias_t[:], 128.0 + float(DELTA2))

            acc_all = constp.tile([128, 16], dt.float32)
            nc.gpsimd.memset(acc_all[:], 0.0)

            for gi in range(n_groups):
                gunits = units[gi * 4:(gi + 1) * 4]
                gw = 512 * len(gunits)
                ps = psp.tile([128, 2048], dt.float32, tag="ps")
                for q, (kind, lp, w) in enumerate(gunits):
                    lhs = znp[:, :, GW + lp * 128:GW + (lp + 1) * 128]
                    if kind == "s":
                        rhs = bkp[:, :, w * 512:(w + 1) * 512]
                    else:
                        rhs = znp[:, :, w * 512:(w + 1) * 512]
                    nc.tensor.matmul(
                        ps[:, q * 512:(q + 1) * 512],
                        lhs, rhs,
                        start=True, stop=True,
                        perf_mode=mybir.MatmulPerfMode.DoubleRow,
                    )
                d8 = d8p.tile([128, 2048], dt.bfloat16, tag="d8")
                nc.scalar.activation(
                    d8[:, 0:gw], ps[:, 0:gw],
                    mybir.ActivationFunctionType.Sqrt,
                    bias=bias_t[:], scale=-128.0,
                )
                et = ep.tile([128, 2048], dt.bfloat16, tag="et")
                mask = r1 if gunits[0][0] == "s" else t2
                mc0 = (gi - n_groups_s) * 2048 if gunits[0][0] == "g" else gi * 2048
                nc.vector.scalar_tensor_tensor(
                    out=et[:, 0:gw],
                    in0=d8[:, 0:gw],
                    scalar=1.0,
                    in1=mask[:, mc0:mc0 + gw],
                    op0=mybir.AluOpType.mult,
                    op1=mybir.AluOpType.mult,
                    accum_out=acc_all[:, gi:gi + 1],
                )

            nc.sync.dma_start(out_d[:], acc_all[:])

    nc.compile()
    return nc, n_groups, n_groups_s


def _get_nc(CS):
    key = ("nc", CS)
    if key not in _CACHE:
        _CACHE[key] = _build_nc(CS)
    return _CACHE[key]


def kernel(y_true, y_pred, lookup, global_step, current_epoch, _want_trace=False):
    from concourse.bass_utils import run_bass_kernel_spmd

    y_true = np.asarray(y_true).astype(np.int64)
    y_pred = np.asarray(y_pred, dtype=np.float32)
    lookup = np.asarray(lookup, dtype=np.float32)
    gs = int(np.asarray(global_step))

    if gs < MOM_WARMUP:
        momentum = 0.5 + (BASE_MOM - 0.5) * (gs / MOM_WARMUP)
    else:
        momentum = BASE_MOM
    progress = min(1.0, (gs - WARMUP_STEPS) / 5000.0)
    aw = MEMORY_WEIGHT * progress

    # ---- host: normalize, bank scatter-EMA, compaction ----
    z = y_pred[:, :D]
    nrm = np.sqrt((z.astype(np.float64) ** 2).sum(axis=1))
    zn = (z / np.maximum(nrm, EPS)[:, None]).astype(np.float32)

    valid = (y_true >= 0) & (y_true < N_CLASSES)
    bg = ~valid
    nv = int(valid.sum())
    lc = np.clip(y_true, 0, N_CLASSES - 1)

    bank = _bank_chains(zn, y_true, momentum)
    init_list = np.array(sorted(bank.keys()), dtype=np.int64)
    C = len(init_list)
    CS = max(1, (C + 511) // 512)
    CP = 512 * CS

    zn_q = cap_fp8(zn)                                      # (B, D)
    bank_rows = (
        np.stack([bank[c] for c in init_list])
        if C else np.zeros((0, D), np.float32)
    )
    bank_q = cap_fp8(bank_rows)                             # (C, D)

    znd = zn_q.astype(np.float64)
    bankd = bank_q.astype(np.float64)

    # ---- host: exact linear terms (fp64) ----
    R = lookup[lc]                    # (B, 8192)
    R_init = R[:, init_list]          # (B, C)
    A_S = 2.0 * nv * C - 2.0 * float(znd[valid].sum(0) @ bankd.sum(0))
    B_S = nv * C - float(R_init[valid].sum(dtype=np.float64))

    T_up = R[:, lc]                   # (B, B): lookup[lc_i, lc_j]
    both_bg = bg[:, None] & bg[None, :]
    one_bg = bg[:, None] ^ bg[None, :]
    T_up = np.where(both_bg, np.float32(BG_SIM),
                    np.where(one_bg, np.float32(BG_OTHER_SIM), T_up))
    # upper-triangle (i<j) oriented pair targets; zero elsewhere
    T_up = np.triu(T_up, 1)

    Np = B * (B - 1) // 2
    szn = znd.sum(0)
    sumG_offdiag = float(szn @ szn) - float((znd ** 2).sum())
    A_G = 2.0 * Np - sumG_offdiag
    B_G = Np - float(T_up.sum(dtype=np.float64))

    # ---- device operand construction ----
    znT = np.ascontiguousarray(zn_q.T)                      # (192, B) fp8
    bankT = np.zeros((D, CP), dtype=f8)
    if C:
        bankT[:, 0:C] = bank_q.T

    bkp_all = np.empty((KP, 2 * CP), dtype=f8)
    bkp_all[:, 0:CP] = bankT[0:KP]
    bkp_all[:, CP:] = bankT[KP:D]

    in_maps = []
    for core in range(NCORES):
        rbs = CORE_RBS[core]
        W = W_A if core < 4 else W_B
        LHS = LHS_A if core < 4 else LHS_B

        # znp: [8 windows x 512 | 6 lhs blocks x 128], 2 k-tiles
        znc = np.empty((D, ZNP_COLS), dtype=f8)
        for w in range(8):
            cc = W[w]
            znc[:, w * 512:(w + 1) * 512] = znT[:, cc * 512:(cc + 1) * 512]
        for lp in range(6):
            rb = rbs[LHS[lp]]
            znc[:, GW + lp * 128:GW + (lp + 1) * 128] = \
                znT[:, rb * 128:rb * 128 + 128]
        znp = np.empty((KP, 2 * ZNP_COLS), dtype=f8)
        znp[:, 0:ZNP_COLS] = znc[0:KP]
        znp[:, ZNP_COLS:] = znc[KP:D]

        r1 = np.zeros((128, 4 * CP), dtype=f8)
        for ib, rb in enumerate(rbs):
            rr = slice(rb * 128, rb * 128 + 128)
            m = (1.0 - R_init[rr]) * valid[rr, None]        # (128, C)
            r1[:, ib * CP:ib * CP + C] = stoch_fp8(m, seed=1000 + rb)

        t2 = np.zeros((128, NGU * 512), dtype=f8)
        for u, (rb, cc) in enumerate(_core_gunits(core)):
            blk = 1.0 - T_up[rb * 128:rb * 128 + 128, cc * 512:(cc + 1) * 512]
            jj = np.arange(cc * 512, cc * 512 + 512)[None, :]
            ii = np.arange(rb * 128, rb * 128 + 128)[:, None]
            blk = np.where(jj > ii, blk, 0.0)
            t2[:, u * 512:(u + 1) * 512] = stoch_fp8(blk, seed=2000 + rb * 8 + cc)

        in_maps.append({
            "znp": znp,
            "bkp": bkp_all,
            "r1": r1,
            "t2": t2,
        })

    nc, n_groups, n_groups_s = _get_nc(CS)
    if _want_trace:
        import tempfile
        try:
            from trn_agent_boot.trn_boot import _ntff_profile_via_ctypes
            hook = _ntff_profile_via_ctypes("/opt/axon/libaxon_pjrt.so")
            outdir = tempfile.mkdtemp(prefix="ntff_")
            with hook(outdir, [0]):
                res = run_bass_kernel_spmd(nc, in_maps, list(range(NCORES)))
            _CACHE["last_profile_dir"] = outdir
        except Exception as e:
            _CACHE["trace_error"] = repr(e)
            res = run_bass_kernel_spmd(nc, in_maps, list(range(NCORES)))
        _CACHE["last_results"] = res
    else:
        res = run_bass_kernel_spmd(nc, in_maps, list(range(NCORES)))

    P3S = 0.0
    P3G = 0.0
    for r in res.results:
        acc = np.asarray(r["acc_out"], dtype=np.float64)
        P3S += float(acc[:, 0:n_groups_s].sum())
        P3G += float(acc[:, n_groups_s:n_groups].sum())

    mem_sum = A_S + 16.0 * B_S - P3S
    denom = max(nv * C, 1)
    mem_loss = mem_sum / denom

    batch_sum = A_G + 16.0 * B_G - P3G
    batch_loss = batch_sum / Np

    loss = (1.0 - aw) * batch_loss + aw * mem_loss
    return np.float32(loss)


# revision 6
# speedup vs baseline: 1.6543x; 1.0094x over previous
"""Trainium2 Bass kernel for ContrastiveAffinityLossWithMemoryV2.

Math: with MARGIN=4 and d = ||a-b|| <= 2 for unit vectors, relu(M-d) = M-d,
so each pairwise loss term simplifies:
    t*d^2 + (1-t)*(M-d)^2 = d^2 + (1-t)*(16 - 8*d)
Sum(d^2) and Sum(1-t) are *linear* and evaluated exactly on host from vector
sums; the only part needing the full B x B pair plane / B x C memory plane is
    P3 = Sum 8*d * (1-t)
which the device computes, sharded over 8 NeuronCores:
  - PE: psum = S via fp8 e4m3 DoubleRow matmuls (K=192 packed as 2 k-tiles of
    96 partitions; one matmul per 128x512 unit at 0.5 cycles/row)
  - ScalarE: d8 = sqrt(-128*psum + 128 + delta) = 8*d (negative scale folds
    the "2-2S" form into the activation's free affine; embeddings/bank rows
    are truncated toward zero in fp8 so every norm stays <= 1 and the sqrt
    argument stays positive)
  - VectorE: scalar_tensor_tensor fused multiply+reduce against host-shipped
    fp8 masks (stochastically rounded so quantization is unbiased), giving
    per-partition partial sums.

The pair plane is computed only for j > i. All cores run ONE program with a
fixed slot pattern; per-core differences live entirely in the data: each
core's znp tensor is [8 rhs windows x 512 cols | 6 lhs blocks x 128 cols]
where the host fills window w with a 512-col chunk and lhs slot p with a
128-row block of its choice. Cores 0-3 have chunk-count profile {8,7,2,1},
cores 4-7 {6,5,4,3}; both embed exactly into the fixed pattern whose
window slot counts are {4,3,2,2,2,2,2,1}.  Host combines device partials
with the closed-form terms.
"""

import numpy as np
import ml_dtypes

N_CLASSES = 8192
B = 4096
D = 192  # 256 * 0.75
KP = 96  # partitions per k-tile (2 tiles of 96 = 192)
NCORES = 8
MARGIN = 4.0
MEMORY_WEIGHT = 0.5
WARMUP_STEPS = 1000
MOM_WARMUP = 5000
BASE_MOM = 0.9
BG_SIM = 0.2
BG_OTHER_SIM = 0.01
EPS = 1e-12
DELTA2 = 0.01
NGU = 18                    # G-plane units per core (144 / 8)
GW = 8 * 512                # G-rhs window region width in znp
ZNP_COLS = GW + 6 * 128     # + 6 lhs block slots

bf16 = ml_dtypes.bfloat16
f8 = ml_dtypes.float8_e4m3

# row-block deal: cores 0-3 get chunk-counts {8,7,2,1}, cores 4-7 {6,5,4,3}
CORE_RBS = [[k, 4 + k, 24 + k, 28 + k] for k in range(4)] + \
           [[8 + k, 12 + k, 16 + k, 20 + k] for k in range(4)]

# fixed G-plane slot pattern: (window, lhs slot), ordered so consecutive
# slots share the same lhs block (stationary reuse on the PE)
P_SLOTS = [
    (0, 0), (1, 0), (2, 0), (4, 0), (6, 0), (7, 0),
    (0, 1), (1, 1), (2, 1), (4, 1), (6, 1),
    (0, 2), (1, 2),
    (0, 3),
    (3, 4), (5, 4),
    (3, 5), (5, 5),
]
# per-core window contents (chunk id per window) and lhs slot contents
# (index into CORE_RBS[core] per lhs slot):
W_A = [7, 6, 1, 2, 3, 4, 5, 0]       # cores 0-3
W_B = [5, 4, 6, 6, 7, 7, 3, 2]       # cores 4-7
LHS_A = [0, 1, 2, 3, 0, 1]
LHS_B = [0, 1, 2, 3, 2, 3]


def _g_chunks(rb):
    """512-col chunks containing any j > i for row-block rb."""
    return [cc for cc in range(8) if 512 * cc + 511 >= 128 * rb + 1]


def _core_gunits(core):
    """(rb, cc) per G slot for this core; asserts global exact cover."""
    rbs = CORE_RBS[core]
    W = W_A if core < 4 else W_B
    LHS = LHS_A if core < 4 else LHS_B
    return [(rbs[LHS[lp]], W[w]) for (w, lp) in P_SLOTS]


def _check_cover():
    seen = []
    for core in range(NCORES):
        seen += _core_gunits(core)
    need = [(rb, cc) for rb in range(32) for cc in _g_chunks(rb)]
    assert sorted(seen) == sorted(need), "G-plane cover mismatch"


_check_cover()

_CACHE = {}


def cap_fp8(v):
    """fp32 -> fp8 e4m3 truncated toward zero: row L2 norms can only shrink."""
    x = np.ascontiguousarray(v, dtype=np.float32)
    y = x.astype(f8)
    yb = y.view(np.uint8).copy()
    over = np.abs(y.astype(np.float32)) > np.abs(x)
    yb[over & ((yb & 0x7F) > 0)] -= 1
    return yb.view(f8)


def stoch_fp8(v, seed):
    """Stochastic rounding to float8_e4m3 (values >= 0)."""
    x = np.ascontiguousarray(v, dtype=np.float32)
    y = x.astype(f8)
    yb = y.view(np.uint8).copy()
    over = np.abs(y.astype(np.float32)) > x
    yb[over & ((yb & 0x7F) > 0)] -= 1
    fl = yb.view(f8)
    ce = (yb + (fl.astype(np.float32) < x).astype(np.uint8)).view(f8)
    flf = fl.astype(np.float32)
    gap = ce.astype(np.float32) - flf
    p = np.where(gap > 0, (x - flf) / np.where(gap > 0, gap, 1.0), 0.0)
    rng = np.random.default_rng(seed)
    up = rng.random(x.shape, dtype=np.float32) < p
    return np.where(up, ce, fl).astype(f8)


def _bank_chains(zn, y_true, momentum):
    """Replicate the reference's sequential per-sample EMA scatter (fp32)."""
    valid = (y_true >= 0) & (y_true < N_CLASSES)
    lc = np.clip(y_true, 0, N_CLASSES - 1)
    m = np.float32(momentum)
    one_m = np.float32(1.0 - momentum)
    bank = {}
    for i in np.nonzero(valid)[0]:
        c = int(lc[i])
        if c not in bank:
            bank[c] = zn[i].copy()
        else:
            ema = m * bank[c] + one_m * zn[i]
            n = np.float32(np.sqrt(np.float32((ema ** 2).sum())))
            bank[c] = ema / max(n, np.float32(EPS))
    return bank


def _groups(CS):
    """Group structure: list of lists of units.

    Units: ("s", lhs slot, bank chunk) | ("g", lhs slot, window).
    First groups are small (512/1536 wide) so the ACT/DVE pipeline starts as
    early as possible; steady state uses 2048-wide groups.
    """
    s_units = [("s", ib, cc) for ib in range(4) for cc in range(CS)]
    g_units = [("g", lp, w) for (w, lp) in P_SLOTS]
    groups = []
    # ib0 split [1, 3, CS-4] for fast pipeline start
    groups.append(s_units[0:1])
    groups.append(s_units[1:4])
    if CS > 4:
        groups.append(s_units[4:CS])
    for ib in range(1, 4):
        base = ib * CS
        groups.append(s_units[base:base + min(4, CS)])
        if CS > 4:
            groups.append(s_units[base + 4:base + CS])
    n_groups_s = len(groups)
    for q in range(0, NGU, 4):
        groups.append(g_units[q:q + 4])
    return groups, n_groups_s


def _build_nc(CS):
    """CS = number of 512-wide S-plane chunks (CP = 512*CS classes)."""
    from concourse import bacc, tile, mybir

    dt = mybir.dt
    CP = 512 * CS
    nc = bacc.Bacc("TRN2", target_bir_lowering=False, debug=False)

    znp_d = nc.dram_tensor("znp", (KP, 2, ZNP_COLS), dt.float8e4, kind="ExternalInput")
    bkp_d = nc.dram_tensor("bkp", (KP, 2, CP), dt.float8e4, kind="ExternalInput")
    r1_d = nc.dram_tensor("r1", (128, 4 * CP), dt.float8e4, kind="ExternalInput")
    t2_d = nc.dram_tensor("t2", (128, NGU * 512), dt.float8e4, kind="ExternalInput")
    out_d = nc.dram_tensor("acc_out", (128, 16), dt.float32, kind="ExternalOutput")

    groups, n_groups_s = _groups(CS)
    n_groups = len(groups)
    assert n_groups <= 16

    with tile.TileContext(nc) as tc:
        with (
            tc.tile_pool(name="const", bufs=1) as constp,
            tc.tile_pool(name="d8p", bufs=3) as d8p,
            tc.tile_pool(name="ep", bufs=2) as ep,
            tc.tile_pool(name="psp", bufs=2, space="PSUM") as psp,
        ):
            # resident operands; [KP, 2, N] with k-tile index in free dim 0
            znp = constp.tile([KP, 2, ZNP_COLS], dt.float8e4, tag="znp")
            bkp = constp.tile([KP, 2, CP], dt.float8e4, tag="bkp")
            r1 = constp.tile([128, 4 * CP], dt.float8e4, tag="r1")
            t2 = constp.tile([128, NGU * 512], dt.float8e4, tag="t2")

            bias_t = constp.tile([128, 1], dt.float32)
            nc.gpsimd.memset(bias_t[:], 128.0 + float(DELTA2))
            warm = constp.tile([128, 1], dt.float32)
            nc.gpsimd.memset(warm[:], 1.0)

            acc_all = constp.tile([128, 16], dt.float32)
            nc.gpsimd.memset(acc_all[:], 0.0)

            # Leading DMAs from the scalar engine (free ~1.2us before SP):
            # lhs blocks, first bank chunk, first mask slice -> group 0 can
            # start while SP streams the rest.
            nc.scalar.dma_start(znp[:, :, GW:ZNP_COLS], znp_d[:, :, GW:ZNP_COLS])
            nc.scalar.dma_start(bkp[:, :, 0:512], bkp_d[:, :, 0:512])
            nc.scalar.dma_start(r1[:, 0:512], r1_d[:, 0:512])
            # preload the sqrt activation table while DMAs stream
            nc.scalar.activation(
                warm[:], warm[:], mybir.ActivationFunctionType.Sqrt, scale=1.0,
            )

            # Remaining input DMAs on SP, in group consumption order.
            nc.sync.dma_start(bkp[:, :, 512:CP], bkp_d[:, :, 512:CP])
            c0 = 512
            for g in range(1, n_groups_s):
                w = 512 * len(groups[g])
                nc.sync.dma_start(r1[:, c0:c0 + w], r1_d[:, c0:c0 + w])
                c0 += w
            nc.sync.dma_start(znp[:, :, 0:GW], znp_d[:, :, 0:GW])
            c0 = 0
            for g in range(n_groups_s, n_groups):
                w = 512 * len(groups[g])
                nc.sync.dma_start(t2[:, c0:c0 + w], t2_d[:, c0:c0 + w])
                c0 += w

            s_off = 0
            g_off = 0
            for gi in range(n_groups):
                gunits = groups[gi]
                gw = 512 * len(gunits)
                ps = psp.tile([128, 2048], dt.float32, tag="ps")
                for q, (kind, lp, w) in enumerate(gunits):
                    lhs = znp[:, :, GW + lp * 128:GW + (lp + 1) * 128]
                    if kind == "s":
                        rhs = bkp[:, :, w * 512:(w + 1) * 512]
                    else:
                        rhs = znp[:, :, w * 512:(w + 1) * 512]
                    nc.tensor.matmul(
                        ps[:, q * 512:(q + 1) * 512],
                        lhs, rhs,
                        start=True, stop=True,
                        perf_mode=mybir.MatmulPerfMode.DoubleRow,
                    )
                d8 = d8p.tile([128, 2048], dt.bfloat16, tag="d8")
                nc.scalar.activation(
                    d8[:, 0:gw], ps[:, 0:gw],
                    mybir.ActivationFunctionType.Sqrt,
                    bias=bias_t[:], scale=-128.0,
                )
                et = ep.tile([128, 2048], dt.bfloat16, tag="et")
                if gunits[0][0] == "s":
                    mask, mc0 = r1, s_off
                    s_off += gw
                else:
                    mask, mc0 = t2, g_off
                    g_off += gw
                nc.vector.scalar_tensor_tensor(
                    out=et[:, 0:gw],
                    in0=d8[:, 0:gw],
                    scalar=1.0,
                    in1=mask[:, mc0:mc0 + gw],
                    op0=mybir.AluOpType.mult,
                    op1=mybir.AluOpType.mult,
                    accum_out=acc_all[:, gi:gi + 1],
                )

            nc.sync.dma_start(out_d[:], acc_all[:])

    nc.compile()
    return nc, n_groups, n_groups_s


def _get_nc(CS):
    key = ("nc", CS)
    if key not in _CACHE:
        _CACHE[key] = _build_nc(CS)
    return _CACHE[key]


def kernel(y_true, y_pred, lookup, global_step, current_epoch, _want_trace=False):
    from concourse.bass_utils import run_bass_kernel_spmd

    y_true = np.asarray(y_true).astype(np.int64)
    y_pred = np.asarray(y_pred, dtype=np.float32)
    lookup = np.asarray(lookup, dtype=np.float32)
    gs = int(np.asarray(global_step))

    if gs < MOM_WARMUP:
        momentum = 0.5 + (BASE_MOM - 0.5) * (gs / MOM_WARMUP)
    else:
        momentum = BASE_MOM
    progress = min(1.0, (gs - WARMUP_STEPS) / 5000.0)
    aw = MEMORY_WEIGHT * progress

    # ---- host: normalize, bank scatter-EMA, compaction ----
    z = y_pred[:, :D]
    nrm = np.sqrt((z.astype(np.float64) ** 2).sum(axis=1))
    zn = (z / np.maximum(nrm, EPS)[:, None]).astype(np.float32)

    valid = (y_true >= 0) & (y_true < N_CLASSES)
    bg = ~valid
    nv = int(valid.sum())
    lc = np.clip(y_true, 0, N_CLASSES - 1)

    bank = _bank_chains(zn, y_true, momentum)
    init_list = np.array(sorted(bank.keys()), dtype=np.int64)
    C = len(init_list)
    CS = max(1, (C + 511) // 512)
    CP = 512 * CS

    zn_q = cap_fp8(zn)                                      # (B, D)
    bank_rows = (
        np.stack([bank[c] for c in init_list])
        if C else np.zeros((0, D), np.float32)
    )
    bank_q = cap_fp8(bank_rows)                             # (C, D)

    znd = zn_q.astype(np.float64)
    bankd = bank_q.astype(np.float64)

    # ---- host: exact linear terms (fp64) ----
    R = lookup[lc]                    # (B, 8192)
    R_init = R[:, init_list]          # (B, C)
    A_S = 2.0 * nv * C - 2.0 * float(znd[valid].sum(0) @ bankd.sum(0))
    B_S = nv * C - float(R_init[valid].sum(dtype=np.float64))

    T_up = R[:, lc]                   # (B, B): lookup[lc_i, lc_j]
    both_bg = bg[:, None] & bg[None, :]
    one_bg = bg[:, None] ^ bg[None, :]
    T_up = np.where(both_bg, np.float32(BG_SIM),
                    np.where(one_bg, np.float32(BG_OTHER_SIM), T_up))
    # upper-triangle (i<j) oriented pair targets; zero elsewhere
    T_up = np.triu(T_up, 1)

    Np = B * (B - 1) // 2
    szn = znd.sum(0)
    sumG_offdiag = float(szn @ szn) - float((znd ** 2).sum())
    A_G = 2.0 * Np - sumG_offdiag
    B_G = Np - float(T_up.sum(dtype=np.float64))

    # ---- device operand construction ----
    znT = np.ascontiguousarray(zn_q.T)                      # (192, B) fp8
    bankT = np.zeros((D, CP), dtype=f8)
    if C:
        bankT[:, 0:C] = bank_q.T

    bkp_all = np.empty((KP, 2, CP), dtype=f8)
    bkp_all[:, 0, :] = bankT[0:KP]
    bkp_all[:, 1, :] = bankT[KP:D]

    in_maps = []
    for core in range(NCORES):
        rbs = CORE_RBS[core]
        W = W_A if core < 4 else W_B
        LHS = LHS_A if core < 4 else LHS_B

        # znp: [8 windows x 512 | 6 lhs blocks x 128], 2 k-tiles
        znc = np.empty((D, ZNP_COLS), dtype=f8)
        for w in range(8):
            cc = W[w]
            znc[:, w * 512:(w + 1) * 512] = znT[:, cc * 512:(cc + 1) * 512]
        for lp in range(6):
            rb = rbs[LHS[lp]]
            znc[:, GW + lp * 128:GW + (lp + 1) * 128] = \
                znT[:, rb * 128:rb * 128 + 128]
        znp = np.empty((KP, 2, ZNP_COLS), dtype=f8)
        znp[:, 0, :] = znc[0:KP]
        znp[:, 1, :] = znc[KP:D]

        r1 = np.zeros((128, 4 * CP), dtype=f8)
        for ib, rb in enumerate(rbs):
            rr = slice(rb * 128, rb * 128 + 128)
            m = (1.0 - R_init[rr]) * valid[rr, None]        # (128, C)
            r1[:, ib * CP:ib * CP + C] = stoch_fp8(m, seed=1000 + rb)

        t2 = np.zeros((128, NGU * 512), dtype=f8)
        for u, (rb, cc) in enumerate(_core_gunits(core)):
            blk = 1.0 - T_up[rb * 128:rb * 128 + 128, cc * 512:(cc + 1) * 512]
            jj = np.arange(cc * 512, cc * 512 + 512)[None, :]
            ii = np.arange(rb * 128, rb * 128 + 128)[:, None]
            blk = np.where(jj > ii, blk, 0.0)
            t2[:, u * 512:(u + 1) * 512] = stoch_fp8(blk, seed=2000 + rb * 8 + cc)

        in_maps.append({
            "znp": znp,
            "bkp": bkp_all,
            "r1": r1,
            "t2": t2,
        })

    nc, n_groups, n_groups_s = _get_nc(CS)
    if _want_trace:
        import tempfile
        try:
            from trn_agent_boot.trn_boot import _ntff_profile_via_ctypes
            hook = _ntff_profile_via_ctypes("/opt/axon/libaxon_pjrt.so")
            outdir = tempfile.mkdtemp(prefix="ntff_")
            with hook(outdir, [0]):
                res = run_bass_kernel_spmd(nc, in_maps, list(range(NCORES)))
            _CACHE["last_profile_dir"] = outdir
        except Exception as e:
            _CACHE["trace_error"] = repr(e)
            res = run_bass_kernel_spmd(nc, in_maps, list(range(NCORES)))
        _CACHE["last_results"] = res
    else:
        res = run_bass_kernel_spmd(nc, in_maps, list(range(NCORES)))

    P3S = 0.0
    P3G = 0.0
    for r in res.results:
        acc = np.asarray(r["acc_out"], dtype=np.float64)
        P3S += float(acc[:, 0:n_groups_s].sum())
        P3G += float(acc[:, n_groups_s:n_groups].sum())

    mem_sum = A_S + 16.0 * B_S - P3S
    denom = max(nv * C, 1)
    mem_loss = mem_sum / denom

    batch_sum = A_G + 16.0 * B_G - P3G
    batch_loss = batch_sum / Np

    loss = (1.0 - aw) * batch_loss + aw * mem_loss
    return np.float32(loss)


# revision 12
# speedup vs baseline: 1.7493x; 1.0574x over previous
"""Trainium2 Bass kernel for ContrastiveAffinityLossWithMemoryV2.

Math: with MARGIN=4 and d = ||a-b|| <= 2 for unit vectors, relu(M-d) = M-d,
so each pairwise loss term simplifies:
    t*d^2 + (1-t)*(M-d)^2 = d^2 + (1-t)*(16 - 8*d)
Sum(d^2) and Sum(1-t) are *linear* and evaluated exactly on host from vector
sums; the only part needing the full B x B pair plane / B x C memory plane is
    P3 = Sum 8*d * (1-t)
which the device computes, sharded over 8 NeuronCores:
  - PE: psum = S via fp8 e4m3 DoubleRow matmuls (K=192 packed as 2 k-tiles of
    96 partitions; one matmul per 128x512 unit at 0.5 cycles/row)
  - ScalarE: d8 = sqrt(-128*psum + 128 + delta) = 8*d (negative scale folds
    the "2-2S" form into the activation's free affine; embeddings/bank rows
    are truncated toward zero in fp8 so every norm stays <= 1 and the sqrt
    argument stays positive)
  - VectorE: scalar_tensor_tensor fused multiply+reduce against host-shipped
    fp8 masks (stochastically rounded so quantization is unbiased), giving
    per-partition partial sums.

The pair plane is computed only for j > i. All cores run ONE program with a
fixed slot pattern; per-core differences live entirely in the data: each
core's znp tensor is [8 rhs windows x 512 cols | 6 lhs blocks x 128 cols]
where the host fills window w with a 512-col chunk and lhs slot p with a
128-row block of its choice. Cores 0-3 have chunk-count profile {8,7,2,1},
cores 4-7 {6,5,4,3}; both embed exactly into the fixed pattern whose
window slot counts are {4,3,2,2,2,2,2,1}.  Host combines device partials
with the closed-form terms.
"""

import numpy as np
import ml_dtypes

N_CLASSES = 8192
B = 4096
D = 192  # 256 * 0.75
KP = 96  # partitions per k-tile (2 tiles of 96 = 192)
NCORES = 8
MARGIN = 4.0
MEMORY_WEIGHT = 0.5
WARMUP_STEPS = 1000
MOM_WARMUP = 5000
BASE_MOM = 0.9
BG_SIM = 0.2
BG_OTHER_SIM = 0.01
EPS = 1e-12
DELTA2 = 0.01
NGU = 18                    # G-plane units per core (144 / 8)
OFFLOAD_GP = False          # run one G group's mask-reduce on GPSIMD
GW = 8 * 512                # G-rhs window region width in znp
ZNP_COLS = GW + 6 * 128     # + 6 lhs block slots

bf16 = ml_dtypes.bfloat16
f8 = ml_dtypes.float8_e4m3

# row-block deal: cores 0-3 get chunk-counts {8,7,2,1}, cores 4-7 {6,5,4,3}
CORE_RBS = [[k, 4 + k, 24 + k, 28 + k] for k in range(4)] + \
           [[8 + k, 12 + k, 16 + k, 20 + k] for k in range(4)]

# fixed G-plane slot pattern: (window, lhs slot), ordered so consecutive
# slots share the same lhs block (stationary reuse on the PE)
P_SLOTS = [
    (0, 0), (1, 0), (2, 0), (4, 0), (6, 0), (7, 0),
    (0, 1), (1, 1), (2, 1), (4, 1), (6, 1),
    (0, 2), (1, 2),
    (0, 3),
    (3, 4), (5, 4),
    (3, 5), (5, 5),
]
# per-core window contents (chunk id per window) and lhs slot contents
# (index into CORE_RBS[core] per lhs slot):
W_A = [7, 6, 1, 2, 3, 4, 5, 0]       # cores 0-3
W_B = [5, 4, 6, 6, 7, 7, 3, 2]       # cores 4-7
LHS_A = [0, 1, 2, 3, 0, 1]
LHS_B = [0, 1, 2, 3, 2, 3]


def _g_chunks(rb):
    """512-col chunks containing any j > i for row-block rb."""
    return [cc for cc in range(8) if 512 * cc + 511 >= 128 * rb + 1]


def _core_gunits(core):
    """(rb, cc) per G slot for this core; asserts global exact cover."""
    rbs = CORE_RBS[core]
    W = W_A if core < 4 else W_B
    LHS = LHS_A if core < 4 else LHS_B
    return [(rbs[LHS[lp]], W[w]) for (w, lp) in P_SLOTS]


def _check_cover():
    seen = []
    for core in range(NCORES):
        seen += _core_gunits(core)
    need = [(rb, cc) for rb in range(32) for cc in _g_chunks(rb)]
    assert sorted(seen) == sorted(need), "G-plane cover mismatch"


_check_cover()

_CACHE = {}


def cap_fp8(v):
    """fp32 -> fp8 e4m3 truncated toward zero: row L2 norms can only shrink."""
    x = np.ascontiguousarray(v, dtype=np.float32)
    y = x.astype(f8)
    yb = y.view(np.uint8).copy()
    over = np.abs(y.astype(np.float32)) > np.abs(x)
    yb[over & ((yb & 0x7F) > 0)] -= 1
    return yb.view(f8)


def stoch_fp8(v, seed):
    """Stochastic rounding to float8_e4m3 (values >= 0)."""
    x = np.ascontiguousarray(v, dtype=np.float32)
    y = x.astype(f8)
    yb = y.view(np.uint8).copy()
    over = np.abs(y.astype(np.float32)) > x
    yb[over & ((yb & 0x7F) > 0)] -= 1
    fl = yb.view(f8)
    ce = (yb + (fl.astype(np.float32) < x).astype(np.uint8)).view(f8)
    flf = fl.astype(np.float32)
    gap = ce.astype(np.float32) - flf
    p = np.where(gap > 0, (x - flf) / np.where(gap > 0, gap, 1.0), 0.0)
    rng = np.random.default_rng(seed)
    up = rng.random(x.shape, dtype=np.float32) < p
    return np.where(up, ce, fl).astype(f8)


def _bank_chains(zn, y_true, momentum):
    """Replicate the reference's sequential per-sample EMA scatter (fp32)."""
    valid = (y_true >= 0) & (y_true < N_CLASSES)
    lc = np.clip(y_true, 0, N_CLASSES - 1)
    m = np.float32(momentum)
    one_m = np.float32(1.0 - momentum)
    bank = {}
    for i in np.nonzero(valid)[0]:
        c = int(lc[i])
        if c not in bank:
            bank[c] = zn[i].copy()
        else:
            ema = m * bank[c] + one_m * zn[i]
            n = np.float32(np.sqrt(np.float32((ema ** 2).sum())))
            bank[c] = ema / max(n, np.float32(EPS))
    return bank


def _groups(CS):
    """Group structure: list of lists of units.

    Units: ("s", lhs slot, bank chunk) | ("g", lhs slot, window).
    A single 1-unit prologue group starts the ACT/DVE pipeline early; all
    other groups are 2048 wide (uniform widths keep the psum double-buffer
    refill fully hidden behind the previous group's activation).
    """
    s_units = [("s", ib, cc) for ib in range(4) for cc in range(CS)]
    g_units = [("g", lp, w) for (w, lp) in P_SLOTS]
    groups = [s_units[0:1]]
    i = 1
    while i < len(s_units):
        groups.append(s_units[i:i + 4])
        i += 4
    n_groups_s = len(groups)
    for q in range(0, NGU, 4):
        groups.append(g_units[q:q + 4])
    return groups, n_groups_s


def _build_nc(CS):
    """CS = number of 512-wide S-plane chunks (CP = 512*CS classes)."""
    from concourse import bacc, tile, mybir

    dt = mybir.dt
    CP = 512 * CS
    nc = bacc.Bacc("TRN2", target_bir_lowering=False, debug=False)

    znp_d = nc.dram_tensor("znp", (KP, 2, ZNP_COLS), dt.float8e4, kind="ExternalInput")
    bkp_d = nc.dram_tensor("bkp", (KP, 2, CP), dt.float8e4, kind="ExternalInput")
    r1_d = nc.dram_tensor("r1", (128, 4 * CP), dt.float8e4, kind="ExternalInput")
    t2_d = nc.dram_tensor("t2", (128, NGU * 512), dt.float8e4, kind="ExternalInput")
    out_d = nc.dram_tensor("acc_out", (128, 16), dt.float32, kind="ExternalOutput")

    groups, n_groups_s = _groups(CS)
    n_groups = len(groups)
    assert n_groups <= 16

    with tile.TileContext(nc) as tc:
        with (
            tc.tile_pool(name="const", bufs=1) as constp,
            tc.tile_pool(name="d8p", bufs=3) as d8p,
            tc.tile_pool(name="ep", bufs=2) as ep,
            tc.tile_pool(name="psp", bufs=2, space="PSUM") as psp,
        ):
            # resident operands; [KP, 2, N] with k-tile index in free dim 0
            znp = constp.tile([KP, 2, ZNP_COLS], dt.float8e4, tag="znp")
            bkp = constp.tile([KP, 2, CP], dt.float8e4, tag="bkp")
            r1 = constp.tile([128, 4 * CP], dt.float8e4, tag="r1")
            t2 = constp.tile([128, NGU * 512], dt.float8e4, tag="t2")

            bias_t = constp.tile([128, 1], dt.float32)
            nc.gpsimd.memset(bias_t[:], 128.0 + float(DELTA2))
            warm = constp.tile([128, 1], dt.float32)
            nc.gpsimd.memset(warm[:], 1.0)

            acc_all = constp.tile([128, 16], dt.float32)
            nc.gpsimd.memset(acc_all[:], 0.0)

            # Leading DMAs from the scalar engine (free ~1.2us before SP):
            # lhs blocks, first bank chunk, first mask slice -> group 0 can
            # start while SP streams the rest.
            nc.scalar.dma_start(znp[:, :, GW:ZNP_COLS], znp_d[:, :, GW:ZNP_COLS])
            nc.scalar.dma_start(bkp[:, :, 0:512], bkp_d[:, :, 0:512])
            nc.scalar.dma_start(r1[:, 0:512], r1_d[:, 0:512])
            # preload the sqrt activation table while DMAs stream
            nc.scalar.activation(
                warm[:], warm[:], mybir.ActivationFunctionType.Sqrt, scale=1.0,
            )

            # Remaining input DMAs on SP, in group consumption order.
            nc.sync.dma_start(r1[:, 512:2560], r1_d[:, 512:2560])
            nc.sync.dma_start(bkp[:, :, 512:2560], bkp_d[:, :, 512:2560])
            nc.sync.dma_start(bkp[:, :, 2560:CP], bkp_d[:, :, 2560:CP])
            c0 = 2560
            for g in range(2, n_groups_s):
                w = 512 * len(groups[g])
                nc.sync.dma_start(r1[:, c0:c0 + w], r1_d[:, c0:c0 + w])
                c0 += w
            nc.sync.dma_start(znp[:, :, 0:GW], znp_d[:, :, 0:GW])
            c0 = 0
            for g in range(n_groups_s, n_groups):
                w = 512 * len(groups[g])
                nc.sync.dma_start(t2[:, c0:c0 + w], t2_d[:, c0:c0 + w])
                c0 += w

            # one G group's mask-reduce runs on the (otherwise idle) GPSIMD
            # engine; dedicated tiles so its slower pace never stalls the
            # d8/et rings used by the DVE groups.
            off_gi = n_groups_s if OFFLOAD_GP else -1
            d8x = constp.tile([128, 2048], dt.bfloat16, tag="d8x")
            etx = constp.tile([128, 2048], dt.bfloat16, tag="etx")

            s_off = 0
            g_off = 0
            for gi in range(n_groups):
                gunits = groups[gi]
                gw = 512 * len(gunits)
                ps = psp.tile([128, 2048], dt.float32, tag="ps")
                for q, (kind, lp, w) in enumerate(gunits):
                    lhs = znp[:, :, GW + lp * 128:GW + (lp + 1) * 128]
                    if kind == "s":
                        rhs = bkp[:, :, w * 512:(w + 1) * 512]
                    else:
                        rhs = znp[:, :, w * 512:(w + 1) * 512]
                    nc.tensor.matmul(
                        ps[:, q * 512:(q + 1) * 512],
                        lhs, rhs,
                        start=True, stop=True,
                        perf_mode=mybir.MatmulPerfMode.DoubleRow,
                    )
                d8 = d8x if gi == off_gi else d8p.tile([128, 2048], dt.bfloat16, tag="d8")
                nc.scalar.activation(
                    d8[:, 0:gw], ps[:, 0:gw],
                    mybir.ActivationFunctionType.Sqrt,
                    bias=bias_t[:], scale=-128.0,
                )
                if gunits[0][0] == "s":
                    mask, mc0 = r1, s_off
                    s_off += gw
                else:
                    mask, mc0 = t2, g_off
                    g_off += gw
                if gi == off_gi:
                    nc.gpsimd.tensor_tensor(
                        etx[:, 0:gw], d8[:, 0:gw], mask[:, mc0:mc0 + gw],
                        mybir.AluOpType.mult,
                    )
                    nc.gpsimd.tensor_reduce(
                        out=acc_all[0:1, 15:16], in_=etx[:, 0:gw],
                        axis=mybir.AxisListType.XYZWC, op=mybir.AluOpType.add,
                    )
                    continue
                et = ep.tile([128, 2048], dt.bfloat16, tag="et")
                nc.vector.scalar_tensor_tensor(
                    out=et[:, 0:gw],
                    in0=d8[:, 0:gw],
                    scalar=1.0,
                    in1=mask[:, mc0:mc0 + gw],
                    op0=mybir.AluOpType.mult,
                    op1=mybir.AluOpType.mult,
                    accum_out=acc_all[:, gi:gi + 1],
                )

            nc.sync.dma_start(out_d[:], acc_all[:])

    nc.compile()
    return nc, n_groups, n_groups_s


def _get_nc(CS):
    key = ("nc", CS)
    if key not in _CACHE:
        _CACHE[key] = _build_nc(CS)
    return _CACHE[key]


def kernel(y_true, y_pred, lookup, global_step, current_epoch, _want_trace=False):
    from concourse.bass_utils import run_bass_kernel_spmd

    y_true = np.asarray(y_true).astype(np.int64)
    y_pred = np.asarray(y_pred, dtype=np.float32)
    lookup = np.asarray(lookup, dtype=np.float32)
    gs = int(np.asarray(global_step))

    if gs < MOM_WARMUP:
        momentum = 0.5 + (BASE_MOM - 0.5) * (gs / MOM_WARMUP)
    else:
        momentum = BASE_MOM
    progress = min(1.0, (gs - WARMUP_STEPS) / 5000.0)
    aw = MEMORY_WEIGHT * progress

    # ---- host: normalize, bank scatter-EMA, compaction ----
    z = y_pred[:, :D]
    nrm = np.sqrt((z.astype(np.float64) ** 2).sum(axis=1))
    zn = (z / np.maximum(nrm, EPS)[:, None]).astype(np.float32)

    valid = (y_true >= 0) & (y_true < N_CLASSES)
    bg = ~valid
    nv = int(valid.sum())
    lc = np.clip(y_true, 0, N_CLASSES - 1)

    bank = _bank_chains(zn, y_true, momentum)
    init_list = np.array(sorted(bank.keys()), dtype=np.int64)
    C = len(init_list)
    CS = max(1, (C + 511) // 512)
    CP = 512 * CS

    zn_q = cap_fp8(zn)                                      # (B, D)
    bank_rows = (
        np.stack([bank[c] for c in init_list])
        if C else np.zeros((0, D), np.float32)
    )
    bank_q = cap_fp8(bank_rows)                             # (C, D)

    znd = zn_q.astype(np.float64)
    bankd = bank_q.astype(np.float64)

    # ---- host: exact linear terms (fp64) ----
    R = lookup[lc]                    # (B, 8192)
    R_init = R[:, init_list]          # (B, C)
    A_S = 2.0 * nv * C - 2.0 * float(znd[valid].sum(0) @ bankd.sum(0))
    B_S = nv * C - float(R_init[valid].sum(dtype=np.float64))

    T_up = R[:, lc]                   # (B, B): lookup[lc_i, lc_j]
    both_bg = bg[:, None] & bg[None, :]
    one_bg = bg[:, None] ^ bg[None, :]
    T_up = np.where(both_bg, np.float32(BG_SIM),
                    np.where(one_bg, np.float32(BG_OTHER_SIM), T_up))
    # upper-triangle (i<j) oriented pair targets; zero elsewhere
    T_up = np.triu(T_up, 1)

    Np = B * (B - 1) // 2
    szn = znd.sum(0)
    sumG_offdiag = float(szn @ szn) - float((znd ** 2).sum())
    A_G = 2.0 * Np - sumG_offdiag
    B_G = Np - float(T_up.sum(dtype=np.float64))

    # ---- device operand construction ----
    znT = np.ascontiguousarray(zn_q.T)                      # (192, B) fp8
    bankT = np.zeros((D, CP), dtype=f8)
    if C:
        bankT[:, 0:C] = bank_q.T

    bkp_all = np.empty((KP, 2, CP), dtype=f8)
    bkp_all[:, 0, :] = bankT[0:KP]
    bkp_all[:, 1, :] = bankT[KP:D]

    in_maps = []
    for core in range(NCORES):
        rbs = CORE_RBS[core]
        W = W_A if core < 4 else W_B
        LHS = LHS_A if core < 4 else LHS_B

        # znp: [8 windows x 512 | 6 lhs blocks x 128], 2 k-tiles
        znc = np.empty((D, ZNP_COLS), dtype=f8)
        for w in range(8):
            cc = W[w]
            znc[:, w * 512:(w + 1) * 512] = znT[:, cc * 512:(cc + 1) * 512]
        for lp in range(6):
            rb = rbs[LHS[lp]]
            znc[:, GW + lp * 128:GW + (lp + 1) * 128] = \
                znT[:, rb * 128:rb * 128 + 128]
        znp = np.empty((KP, 2, ZNP_COLS), dtype=f8)
        znp[:, 0, :] = znc[0:KP]
        znp[:, 1, :] = znc[KP:D]

        r1 = np.zeros((128, 4 * CP), dtype=f8)
        for ib, rb in enumerate(rbs):
            rr = slice(rb * 128, rb * 128 + 128)
            m = (1.0 - R_init[rr]) * valid[rr, None]        # (128, C)
            r1[:, ib * CP:ib * CP + C] = stoch_fp8(m, seed=1000 + rb)

        t2 = np.zeros((128, NGU * 512), dtype=f8)
        for u, (rb, cc) in enumerate(_core_gunits(core)):
            blk = 1.0 - T_up[rb * 128:rb * 128 + 128, cc * 512:(cc + 1) * 512]
            jj = np.arange(cc * 512, cc * 512 + 512)[None, :]
            ii = np.arange(rb * 128, rb * 128 + 128)[:, None]
            blk = np.where(jj > ii, blk, 0.0)
            t2[:, u * 512:(u + 1) * 512] = stoch_fp8(blk, seed=2000 + rb * 8 + cc)

        in_maps.append({
            "znp": znp,
            "bkp": bkp_all,
            "r1": r1,
            "t2": t2,
        })

    nc, n_groups, n_groups_s = _get_nc(CS)
    if _want_trace:
        import tempfile
        try:
            from trn_agent_boot.trn_boot import _ntff_profile_via_ctypes
            hook = _ntff_profile_via_ctypes("/opt/axon/libaxon_pjrt.so")
            outdir = tempfile.mkdtemp(prefix="ntff_")
            with hook(outdir, [0]):
                res = run_bass_kernel_spmd(nc, in_maps, list(range(NCORES)))
            _CACHE["last_profile_dir"] = outdir
        except Exception as e:
            _CACHE["trace_error"] = repr(e)
            res = run_bass_kernel_spmd(nc, in_maps, list(range(NCORES)))
        _CACHE["last_results"] = res
    else:
        res = run_bass_kernel_spmd(nc, in_maps, list(range(NCORES)))

    P3S = 0.0
    P3G = 0.0
    for r in res.results:
        acc = np.asarray(r["acc_out"], dtype=np.float64)
        P3S += float(acc[:, 0:n_groups_s].sum())
        P3G += float(acc[:, n_groups_s:n_groups].sum()) + float(acc[0, 15])

    mem_sum = A_S + 16.0 * B_S - P3S
    denom = max(nv * C, 1)
    mem_loss = mem_sum / denom

    batch_sum = A_G + 16.0 * B_G - P3G
    batch_loss = batch_sum / Np

    loss = (1.0 - aw) * batch_loss + aw * mem_loss
    return np.float32(loss)
